# revision 3
# baseline (speedup 1.0000x reference)
"""Window-routed sparse attention on 8 TRN2 NeuronCores.

Sharding: 64 windows x 8 cores = 8 windows/core (embarrassingly parallel).
Host precomputes the tiny routing path (region means, a_r [64,64]) and the
window-mixed q_m/k_m/v in fp32 numpy; each core runs the heavy windowed
attention relu(q_m k_m^T) v for its 8 windows on the Tensor engine in
float32r (full-rate fp32).
"""

import sys

sys.path.insert(0, "/opt/trn_rl_repo")

import numpy as np

C = 64          # channels
NW = 64         # windows (8x8 grid of 32x32 patches on 256x256)
T = 1024        # tokens per window (32*32)
NCORES = 8
WPC = NW // NCORES  # windows per core

_CACHE = {}


def _build_program():
    import concourse.mybir as mybir
    from concourse import bacc
    from concourse.tile import TileContext

    f32r = mybir.dt.float32r
    f32 = mybir.dt.float32

    nc = bacc.Bacc(None, target_bir_lowering=False)
    # c-major [c, i, t] for q_m/k_m; token-major [i, k, p, c] for v
    qm_d = nc.declare_dram_parameter("qm", [C, WPC, T], f32r, isOutput=False)
    km_d = nc.declare_dram_parameter("km", [C, WPC, T], f32r, isOutput=False)
    v_d = nc.declare_dram_parameter("v", [WPC, 8, 128, C], f32r, isOutput=False)
    o_d = nc.declare_dram_parameter("o", [C, WPC, T], f32r, isOutput=True)

    with TileContext(nc) as tc:
        with (
            tc.tile_pool(name="qk", bufs=2) as qk_pool,
            tc.tile_pool(name="vp", bufs=2) as v_pool,
            tc.tile_pool(name="at", bufs=2) as a_pool,
            tc.tile_pool(name="ob", bufs=2) as o_pool,
            tc.tile_pool(name="pa", bufs=2, space="PSUM") as pa_pool,
            tc.tile_pool(name="po", bufs=2, space="PSUM") as po_pool,
        ):
            for i in range(WPC):
                qm_t = qk_pool.tile([C, T], f32r, tag="qm")
                km_t = qk_pool.tile([C, T], f32r, tag="km")
                v_t = v_pool.tile([128, 8, C], f32r, tag="v")
                nc.sync.dma_start(out=qm_t, in_=qm_d[:, i, :])
                nc.sync.dma_start(out=km_t, in_=km_d[:, i, :])
                nc.sync.dma_start(out=v_t, in_=v_d[i].rearrange("k p c -> p k c"))

                # fused per s-chunk: qk matmul -> relu -> o accumulate.
                # single relu engine keeps per-instruction sync waits low.
                ps_o = po_pool.tile([C, T], f32, tag="pso")
                for k in range(8):
                    ps_a = pa_pool.tile([128, T], f32, tag="psa")
                    for h in range(2):
                        nc.tensor.matmul(
                            out=ps_a[:, h * 512:(h + 1) * 512],
                            lhsT=km_t[:, k * 128:(k + 1) * 128],
                            rhs=qm_t[:, h * 512:(h + 1) * 512],
                            start=True,
                            stop=True,
                        )
                    attn_t = a_pool.tile([128, T], f32r, tag="attn")
                    nc.scalar.activation(
                        out=attn_t,
                        in_=ps_a,
                        func=mybir.ActivationFunctionType.Relu,
                        scale=1.0,
                    )
                    for h in range(2):
                        nc.tensor.matmul(
                            out=ps_o[:, h * 512:(h + 1) * 512],
                            lhsT=v_t[:, k, :],
                            rhs=attn_t[:, h * 512:(h + 1) * 512],
                            start=(k == 0),
                            stop=(k == 7),
                        )
                o_t = o_pool.tile([C, T], f32r, tag="o")
                nc.vector.tensor_copy(out=o_t, in_=ps_o)
                nc.sync.dma_start(out=o_d[:, i, :], in_=o_t)

    nc.finalize()
    return nc


LAST_RESULT = None  # BassKernelResults from the most recent run (for test.py)


def kernel(x, W, bias, _trace=False):
    global LAST_RESULT
    from concourse.bass_utils import run_bass_kernel_spmd

    x = np.asarray(x, dtype=np.float32)
    W = np.asarray(W, dtype=np.float32)
    bias = np.asarray(bias, dtype=np.float32)

    # ---- host prep: windows, qkv, routing, mixing (tiny vs attention) ----
    # xw: [nw, T, c]
    xw = (
        x.reshape(C, 8, 32, 8, 32)
        .transpose(1, 3, 2, 4, 0)
        .reshape(NW, T, C)
    )
    qkv = xw @ W.T + bias  # [nw, T, 3c]
    q, k, v = qkv[..., :C], qkv[..., C:2 * C], qkv[..., 2 * C:]
    q_r = q.mean(axis=1)  # [nw, c]
    k_r = k.mean(axis=1)
    a_r = np.maximum(q_r @ k_r.T, 0.0)  # [nw, nw]
    k_m = np.tensordot(a_r, k, axes=(1, 0))  # [nw, T, c]
    q_m = np.tensordot(a_r, q, axes=(1, 0))

    if "nc" not in _CACHE:
        _CACHE["nc"] = _build_program()
    nc = _CACHE["nc"]

    in_maps = []
    for m in range(NCORES):
        s = slice(m * WPC, (m + 1) * WPC)
        in_maps.append({
            "qm": np.ascontiguousarray(q_m[s].transpose(2, 0, 1)),  # [c, wpc, T]
            "km": np.ascontiguousarray(k_m[s].transpose(2, 0, 1)),
            "v": np.ascontiguousarray(v[s].reshape(WPC, 8, 128, C)),
        })

    res = run_bass_kernel_spmd(nc, in_maps, list(range(NCORES)), trace=_trace)
    LAST_RESULT = res
    outs = [res.results[m]["o"].reshape(C, WPC, T) for m in range(NCORES)]
    o_cm = np.concatenate(outs, axis=1)  # [c, nw, T]

    # fold back: [c, jh, jw, th, tw] -> [1, c, 256, 256]
    o_img = (
        o_cm.reshape(C, 8, 8, 32, 32)
        .transpose(0, 1, 3, 2, 4)
        .reshape(1, C, 256, 256)
    )
    return o_img.astype(np.float32)



# revision 5
# speedup vs baseline: 1.6667x; 1.6667x over previous
"""Window-routed sparse attention on 8 TRN2 NeuronCores.

Sharding: 64 windows x 8 cores = 8 windows/core (embarrassingly parallel).
Host precomputes the tiny routing path (region means, a_r [64,64]) and the
window-mixed q_m/k_m in fp32; each core runs the heavy windowed attention
relu(q_m k_m^T) v for its 8 windows on the Tensor engine in bf16 (f32 PSUM
accumulation). Relu alternates between the Scalar and Vector engines so it
hides under the matmuls; PSUM->SBUF output copies run on GpSimd.
"""

import sys

sys.path.insert(0, "/opt/trn_rl_repo")

import numpy as np
import ml_dtypes

C = 64          # channels
NW = 64         # windows (8x8 grid of 32x32 patches on 256x256)
T = 1024        # tokens per window (32*32)
NCORES = 8
WPC = NW // NCORES  # windows per core
BF16 = ml_dtypes.bfloat16

_CACHE = {}


def _build_program():
    import concourse.mybir as mybir
    from concourse import bacc
    from concourse.tile import TileContext

    bf16 = mybir.dt.bfloat16
    f32 = mybir.dt.float32

    nc = bacc.Bacc(None, target_bir_lowering=False)
    # c-major [c, i, t] for q_m/k_m; [s, i, k, c] for v (s = token % 128,
    # k = token // 128 within the window)
    qm_d = nc.declare_dram_parameter("qm", [C, WPC, T], bf16, isOutput=False)
    km_d = nc.declare_dram_parameter("km", [C, WPC, T], bf16, isOutput=False)
    v_d = nc.declare_dram_parameter("v", [128, WPC, 8, C], bf16, isOutput=False)
    o_d = nc.declare_dram_parameter("o", [C, WPC, T], bf16, isOutput=True)

    with TileContext(nc) as tc:
        with (
            tc.tile_pool(name="in", bufs=1) as in_pool,
            tc.tile_pool(name="at", bufs=2) as a_pool,
            tc.tile_pool(name="ob", bufs=2) as o_pool,
            tc.tile_pool(name="pa", bufs=2, space="PSUM") as pa_pool,
            tc.tile_pool(name="po", bufs=2, space="PSUM") as po_pool,
        ):
            # persistent SBUF tiles, loaded once (DMA split per window so
            # the pieces spread across queues and window 0 starts early)
            qm_t = in_pool.tile([C, WPC, T], bf16, tag="qm")
            km_t = in_pool.tile([C, WPC, T], bf16, tag="km")
            v_t = in_pool.tile([128, WPC, 8, C], bf16, tag="v")
            o_t = in_pool.tile([C, WPC, T], bf16, tag="o")
            for i in range(WPC):
                nc.sync.dma_start(out=qm_t[:, i], in_=qm_d[:, i])
                nc.sync.dma_start(out=km_t[:, i], in_=km_d[:, i])
                nc.sync.dma_start(out=v_t[:, i], in_=v_d[:, i])

            for i in range(WPC):
                # per s-chunk k: QK matmul -> relu -> AV accumulate into ps_o
                ps_o = po_pool.tile([C, T], f32, tag="pso")
                for k in range(8):
                    ps_a = pa_pool.tile([128, T], f32, tag="psa")
                    for h in range(2):
                        nc.tensor.matmul(
                            out=ps_a[:, h * 512:(h + 1) * 512],
                            lhsT=km_t[:, i, k * 128:(k + 1) * 128],
                            rhs=qm_t[:, i, h * 512:(h + 1) * 512],
                            start=True,
                            stop=True,
                        )
                    attn_t = a_pool.tile([128, T], bf16, tag="attn")
                    if k % 2 == 0:
                        nc.scalar.activation(
                            out=attn_t,
                            in_=ps_a,
                            func=mybir.ActivationFunctionType.Relu,
                            scale=1.0,
                        )
                    else:
                        nc.vector.tensor_scalar_max(attn_t, ps_a, 0.0)
                    for h in range(2):
                        nc.tensor.matmul(
                            out=ps_o[:, h * 512:(h + 1) * 512],
                            lhsT=v_t[:, i, k, :],
                            rhs=attn_t[:, h * 512:(h + 1) * 512],
                            start=(k == 0),
                            stop=(k == 7),
                        )
                if i % 2 == 0:
                    nc.vector.tensor_copy(out=o_t[:, i], in_=ps_o)
                else:
                    nc.scalar.activation(
                        out=o_t[:, i],
                        in_=ps_o,
                        func=mybir.ActivationFunctionType.Copy,
                        scale=1.0,
                    )
                nc.sync.dma_start(out=o_d[:, i], in_=o_t[:, i])

    nc.finalize()
    return nc


def kernel(x, W, bias, _trace=False):
    global LAST_RESULT
    from concourse.bass_utils import run_bass_kernel_spmd

    x = np.asarray(x, dtype=np.float32)
    W = np.asarray(W, dtype=np.float32)
    bias = np.asarray(bias, dtype=np.float32)

    # ---- host prep: windows, qkv, routing, mixing (tiny vs attention) ----
    # xw: [nw, T, c]
    xw = (
        x.reshape(C, 8, 32, 8, 32)
        .transpose(1, 3, 2, 4, 0)
        .reshape(NW, T, C)
    )
    qkv = xw @ W.T + bias  # [nw, T, 3c]
    q, k, v = qkv[..., :C], qkv[..., C:2 * C], qkv[..., 2 * C:]
    q_r = q.mean(axis=1)  # [nw, c]
    k_r = k.mean(axis=1)
    a_r = np.maximum(q_r @ k_r.T, 0.0)  # [nw, nw]
    k_m = np.tensordot(a_r, k, axes=(1, 0))  # [nw, T, c]
    q_m = np.tensordot(a_r, q, axes=(1, 0))

    if "nc" not in _CACHE:
        _CACHE["nc"] = _build_program()
    nc = _CACHE["nc"]

    in_maps = []
    for m in range(NCORES):
        s = slice(m * WPC, (m + 1) * WPC)
        # v: [wpc, T, c] -> [wpc, k, s(128), c] -> [s, wpc, k, c]
        v_s = v[s].reshape(WPC, 8, 128, C).transpose(2, 0, 1, 3)
        in_maps.append({
            "qm": np.ascontiguousarray(q_m[s].transpose(2, 0, 1)).astype(BF16),
            "km": np.ascontiguousarray(k_m[s].transpose(2, 0, 1)).astype(BF16),
            "v": np.ascontiguousarray(v_s).astype(BF16),
        })

    res = run_bass_kernel_spmd(nc, in_maps, list(range(NCORES)), trace=_trace)
    LAST_RESULT = res
    outs = [
        res.results[m]["o"].astype(np.float32).reshape(C, WPC, T)
        for m in range(NCORES)
    ]
    o_cm = np.concatenate(outs, axis=1)  # [c, nw, T]

    # fold back: [c, jh, jw, th, tw] -> [1, c, 256, 256]
    o_img = (
        o_cm.reshape(C, 8, 8, 32, 32)
        .transpose(0, 1, 3, 2, 4)
        .reshape(1, C, 256, 256)
    )
    return o_img.astype(np.float32)


LAST_RESULT = None  # BassKernelResults from the most recent run (for test.py)
